# revision 1
# baseline (speedup 1.0000x reference)
"""Trainium2 Bass kernel for greedy sequential independent-set sampling.

Reference semantics: sites visited in row-major order; site (r, c) is set to 1
iff u[s, r, c] < 0.5 and no already-set lattice neighbor. Because the visit
order is row-major, right/down neighbors are still 0 when a site is decided:

    x[r, c] = (u[r, c] < 0.5) & ~x[r-1, c] & ~x[r, c-1]

One DVE tensor_tensor_scan per lattice row computes the whole thing:

    state' = (b[c] - state) is_gt x_prev[c]      (op0=subtract, op1=is_gt)

where b = (u < 0.5) in {0,1} is computed ON THE HOST (it's a pure input
transform) and state carries x[r, c-1]: b - x_left = 1 iff bernoulli hit AND
left free; > x_up iff up free. A dummy b=0 column per 32-col group resets the
carried state at group boundaries, so many samples pack side by side in one
scan's free dim.

Device program per core (65536 samples -> 8 cores x 8192 = 64 groups of 128):
  - 32 chained DVE scans over [P, 64*33] rows (the scan is DVE-only:
    neuronxcc rejects TensorTensorScanArith on Pool),
  - input b bytes DMA'd in on the SP queue (sliced so row 0 lands early),
  - finished x rows DMA'd out as RAW int8 (dummy cols included) on the
    Activation engine's queue; the host strips dummies, reorders, and
    widens to int32. No ScalarE/Pool compute at all on the device.
"""

import numpy as np

import concourse.bacc as bacc
import concourse.mybir as mybir
from concourse.tile import TileContext
from concourse.bass_utils import run_bass_kernel_spmd

N_CORES = 8
S_TOTAL = 65536
R = 32
C = 32
P = 128  # SBUF partitions

SPC = S_TOTAL // N_CORES  # samples per core: 8192
G = SPC // P  # 64 groups of 128 samples
W = C + 1  # 33: one dummy col per group resets the scan carry
L = G * W  # 2112 bytes per lattice row

F32 = mybir.dt.float32
I8 = mybir.dt.int8


def build_nc():
    """Build the per-core Bass program (SPMD: same program, different data)."""
    nc = bacc.Bacc("TRN2", target_bir_lowering=False, debug=False)
    u_in = nc.declare_dram_parameter("u", [P, R * L], I8, isOutput=False)
    cfg = nc.declare_dram_parameter("config", [P, R * L], I8, isOutput=True)

    with TileContext(nc) as tc:
        with tc.tile_pool(name="bufs", bufs=1) as pool:
            nb = pool.tile([P, R * L], I8, tag="nb")
            x = pool.tile([P, (R + 1) * L], I8, tag="x")

            # x row -1 = zeros
            nc.vector.memset(x[:, 0:L], 0)

            # Input: row 0 lands in ~1 us so the scan chain starts early;
            # later rows stream in 4-row (~8 KiB/partition) pieces.
            segs = [1, 1, 2, 4] + [4] * 6
            a = 0
            for nr in segs:
                nc.sync.dma_start(
                    out=nb[:, a * L : (a + nr) * L],
                    in_=u_in[:, a * L : (a + nr) * L],
                )
                a += nr

            # Output: DMA finished rows straight from x (int8, dummies
            # included). A block's reads must stay a full SBUF bank (2 KiB)
            # behind the next scan's write, or Tile's bank-level WAR
            # tracking serializes the scan chain -> lag in rows. Issued on
            # the Activation engine's DGE queue so output never queues
            # behind input on SP's.
            blocks = [(b0, 4) for b0 in range(0, R - 4, 4)]
            blocks += [(R - 4, 2), (R - 2, 1), (R - 1, 1)]
            LAG = max(2, -(-2048 // L))
            bi = 0

            for r in range(R):
                # state' = (b - state) > x_up : the full site update
                nc.vector.tensor_tensor_scan(
                    out=x[:, (r + 1) * L : (r + 2) * L],
                    data0=nb[:, r * L : (r + 1) * L],
                    data1=x[:, r * L : (r + 1) * L],
                    initial=0.0,
                    op0=mybir.AluOpType.subtract,
                    op1=mybir.AluOpType.is_gt,
                )
                while bi < len(blocks) and (
                    r + 1 >= blocks[bi][0] + blocks[bi][1] + LAG
                ):
                    b0, nr = blocks[bi]
                    nc.scalar.dma_start(
                        out=cfg[:, b0 * L : (b0 + nr) * L],
                        in_=x[:, (b0 + 1) * L : (b0 + 1 + nr) * L],
                    )
                    bi += 1
            while bi < len(blocks):
                b0, nr = blocks[bi]
                nc.scalar.dma_start(
                    out=cfg[:, b0 * L : (b0 + nr) * L],
                    in_=x[:, (b0 + 1) * L : (b0 + 1 + nr) * L],
                )
                bi += 1
    nc.compile()
    return nc


def prep_core(b_core):
    """[SPC, 32, 32] int8 {0,1} -> device layout [P, R*G*W].

    Layout [p][r][g][w] with b=0 at w=0 of each group (the dummy column
    that resets the scan carry); sample s = g*P + p.
    """
    v = b_core.reshape(G, P, R, C).transpose(1, 2, 0, 3)  # [P, R, G, C]
    out = np.zeros((P, R, G, W), np.int8)
    out[..., 1:] = v
    return {"u": out.reshape(P, R * L)}


def assemble_core(res_map):
    """Device output -> [SPC, 32, 32] int8."""
    v = res_map["config"].reshape(P, R, G, W)[..., 1:]  # [P, R, G, C]
    return np.ascontiguousarray(v.transpose(2, 0, 1, 3)).reshape(SPC, R, C)


def host_prep_all(u):
    """Full u -> per-core in_maps. b = (u < 0.5) via the fp32 top byte."""
    b3 = np.ascontiguousarray(u, dtype=np.float32).reshape(-1).view(np.uint8)[3::4]
    b = (b3 < 63).view(np.int8).reshape(S_TOTAL, R, C)
    return [prep_core(b[i * SPC : (i + 1) * SPC]) for i in range(N_CORES)]


_NC_CACHE = {}


def _get_nc():
    if "nc" not in _NC_CACHE:
        _NC_CACHE["nc"] = build_nc()
    return _NC_CACHE["nc"]


def kernel(u, n_rows=32, n_cols=32, **_):
    u = np.asarray(u)
    assert u.shape == (S_TOTAL, R, C), u.shape
    assert int(n_rows) == R and int(n_cols) == C

    nc = _get_nc()
    in_maps = host_prep_all(u)
    res = run_bass_kernel_spmd(nc, in_maps, list(range(N_CORES)))
    out = np.concatenate(
        [assemble_core(res.results[i]) for i in range(N_CORES)], axis=0
    )
    return out.astype(np.int32).reshape(S_TOTAL, R, C)



# revision 7
# speedup vs baseline: 2.5912x; 2.5912x over previous
"""Trainium2 Bass kernel for greedy sequential independent-set sampling.

Reference semantics: sites visited in row-major order; site (r, c) is set to 1
iff u[s, r, c] < 0.5 and no already-set lattice neighbor. Row-major order means
right/down neighbors are still 0 when a site is decided:

    x[r, c] = b[r, c] & ~x[r-1, c] & ~x[r, c-1],   b = (u < 0.5)

Pack one sample's 32-col lattice row into ONE int32 word (bit c = col c).
With a = b & ~x_up, the left-neighbor recurrence x_c = a_c & ~x_{c-1} is a
two-state automaton along the bits — exactly an adder carry chain. Writing
y_c = x_c ^ m_c with m = 0x55555555 turns it into the carry recurrence of
the sum  S = m + (a ^ m):

    x = a & (((a ^ S) >> 1) ^ m)        (>> = arithmetic shift: the sign
                                         bit supplies the top column)

so a whole 32-site row scan collapses into a handful of bitwise ops plus one
integer add. DVE does all bitwise/shift work (fused 2-op instructions); the
add runs on Pool, whose int32 adder is exact mod 2^32 (DVE's is float-based).

Per-core per row r (state na = ~a, chain state nx = ~x of previous row):
    na = (nx ^ -1) | bn        DVE scalar_tensor_tensor   (bn = ~b from host)
    t  = na ^ 0xAAAAAAAA       DVE tensor_scalar          (t = a ^ m)
    S  = t + 0x55555555        Pool tensor_tensor add     (exact, wraps)
    h  = (na >>a 1) ^ m        DVE tensor_scalar          (overlaps Pool add)
    g  = (S >>a 1) ^ h         DVE scalar_tensor_tensor   (g = ~x's upper part)
    nx = g | na                DVE tensor_tensor          (nx = ~x, DMA'd out)

The mask m must be applied AFTER the shift (positions 30 and 31 both read
bit 31 under arithmetic shift but need opposite mask parity), hence h/g.

Host packs b bits into words (bn = ~b), unpacks ~nx at the end. 65536 samples
-> 8 cores x 8192; 8192 samples = 64 words x 128 partitions per row step.
"""

import numpy as np

import concourse.bacc as bacc
import concourse.mybir as mybir
from concourse.tile import TileContext
from concourse.bass_utils import run_bass_kernel_spmd

N_CORES = 8
S_TOTAL = 65536
R = 32
C = 32
P = 128  # SBUF partitions

SPC = S_TOTAL // N_CORES  # samples per core: 8192
G = SPC // P  # 64 words per partition per row step
NW = R * G  # 2048 int32 words per partition

I32 = mybir.dt.int32
ALU = mybir.AluOpType

M = 0x55555555  # bits at even columns
NM = 0xAAAAAAAA  # ~M
ALL1 = 0xFFFFFFFF


def _s32(v):
    v &= 0xFFFFFFFF
    return v - (1 << 32) if v >= (1 << 31) else v


def _imm(v):
    return mybir.ImmediateValue(dtype=I32, value=_s32(v))


def _stt(eng, out, in0, scalar, in1, op0, op1):
    """out = (in0 op0 scalar) op1 in1 with an int32 immediate."""
    return eng.add_instruction(
        mybir.InstTensorScalarPtr(
            name=eng.bass.get_next_instruction_name(),
            is_scalar_tensor_tensor=True,
            op0=op0,
            op1=op1,
            ins=[eng.lower_ap(in0), _imm(scalar), eng.lower_ap(in1)],
            outs=[eng.lower_ap(out)],
        )
    )


def _ts(eng, out, in0, s1, op0, s2=None, op1=None):
    """out = (in0 op0 s1) [op1 s2] with int32 immediates."""
    ins = [eng.lower_ap(in0), _imm(s1)]
    kw = dict(op0=op0)
    if op1 is not None:
        ins.append(_imm(s2))
        kw["op1"] = op1
    return eng.add_instruction(
        mybir.InstTensorScalarPtr(
            name=eng.bass.get_next_instruction_name(),
            ins=ins,
            outs=[eng.lower_ap(out)],
            **kw,
        )
    )


def build_nc():
    """Build the per-core Bass program (SPMD: same program, different data)."""
    nc = bacc.Bacc("TRN2", target_bir_lowering=False, debug=False)
    bn_in = nc.declare_dram_parameter("bn", [P, NW], I32, isOutput=False)
    out = nc.declare_dram_parameter("out", [P, NW], I32, isOutput=True)

    dve = nc.vector
    pl = nc.gpsimd  # Pool engine: exact int32 adds

    with TileContext(nc) as tc:
        with tc.tile_pool(name="bufs", bufs=1) as pool:
            bn = pool.tile([P, NW], I32, tag="bn")
            nxo = pool.tile([P, NW], I32, tag="nxo")
            mt = pool.tile([P, G], I32, tag="mt")
            na = [pool.tile([P, G], I32, name=f"na{i}", tag=f"na{i}") for i in range(2)]
            tt = [pool.tile([P, G], I32, name=f"t{i}", tag=f"t{i}") for i in range(2)]
            ss = [pool.tile([P, G], I32, name=f"s{i}", tag=f"s{i}") for i in range(2)]
            hh = [pool.tile([P, G], I32, name=f"h{i}", tag=f"h{i}") for i in range(2)]
            gg = [pool.tile([P, G], I32, name=f"g{i}", tag=f"g{i}") for i in range(2)]

            pl.memset(mt[:], _s32(M))

            # Input: row 0 lands early so the chain starts; later rows
            # stream in bigger pieces on the SP queue.
            segs = [1, 1, 2, 4] + [4] * 6
            a = 0
            for nr in segs:
                nc.sync.dma_start(
                    out=bn[:, a * G : (a + nr) * G],
                    in_=bn_in[:, a * G : (a + nr) * G],
                )
                a += nr

            for r in range(R):
                i = r & 1
                if r == 0:
                    na_r = bn[:, 0:G]  # nx_init = all ones -> na = bn row 0
                else:
                    na_r = na[i][:]
                    _stt(
                        dve, na_r, nxo[:, (r - 1) * G : r * G], ALL1,
                        bn[:, r * G : (r + 1) * G],
                        ALU.bitwise_xor, ALU.bitwise_or,
                    )
                _ts(dve, tt[i][:], na_r, NM, ALU.bitwise_xor)
                pl.tensor_tensor(out=ss[i][:], in0=tt[i][:], in1=mt[:], op=ALU.add)
                _ts(dve, hh[i][:], na_r, 1, ALU.arith_shift_right,
                    M, ALU.bitwise_xor)
                _stt(dve, gg[i][:], ss[i][:], 1, hh[i][:],
                     ALU.arith_shift_right, ALU.bitwise_xor)
                dve.tensor_tensor(
                    out=nxo[:, r * G : (r + 1) * G], in0=gg[i][:], in1=na_r,
                    op=ALU.bitwise_or,
                )

                # Drain finished rows in 8-row (one SBUF bank) batches on
                # the Activation engine's DGE queue.
                if r % 8 == 7:
                    b0 = r - 7
                    nc.scalar.dma_start(
                        out=out[:, b0 * G : (r + 1) * G],
                        in_=nxo[:, b0 * G : (r + 1) * G],
                    )
    nc.compile()
    return nc


def host_prep_all(u):
    """Full u -> per-core in_maps of packed ~b words, layout [p, r*G+g]."""
    b3 = np.ascontiguousarray(u, dtype=np.float32).reshape(-1).view(np.uint8)[3::4]
    bits = (b3 < 63).astype(np.uint8).reshape(S_TOTAL, R, C)
    bw = np.packbits(bits, axis=-1, bitorder="little")  # [S, R, 4] bytes
    bn = ~(bw.reshape(S_TOTAL, R * 4).view(np.uint32))  # [S, R] words, ~b
    maps = []
    for k in range(N_CORES):
        w = bn[k * SPC : (k + 1) * SPC]  # [8192, 32], s = g*P + p
        dev = w.reshape(G, P, R).transpose(1, 2, 0)  # [P, R, G]
        maps.append({"bn": np.ascontiguousarray(dev).reshape(P, NW).view(np.int32)})
    return maps


def assemble_core(res_map):
    """Device output (nx words) -> [SPC, 32, 32] uint8 {0,1}."""
    nx = res_map["out"].view(np.uint32).reshape(P, R, G)
    xw = (~nx).transpose(2, 0, 1).reshape(SPC, R)  # [s, r] words, s = g*P+p
    xb = np.ascontiguousarray(xw).view(np.uint8).reshape(SPC, R, 4)
    return np.unpackbits(xb, axis=-1, bitorder="little")  # [SPC, R, 32]


_NC_CACHE = {}


def _get_nc():
    if "nc" not in _NC_CACHE:
        _NC_CACHE["nc"] = build_nc()
    return _NC_CACHE["nc"]


def kernel(u, n_rows=32, n_cols=32, **_):
    u = np.asarray(u)
    assert u.shape == (S_TOTAL, R, C), u.shape
    assert int(n_rows) == R and int(n_cols) == C

    nc = _get_nc()
    in_maps = host_prep_all(u)
    res = run_bass_kernel_spmd(nc, in_maps, list(range(N_CORES)))
    out = np.concatenate(
        [assemble_core(res.results[i]) for i in range(N_CORES)], axis=0
    )
    return out.astype(np.int32).reshape(S_TOTAL, R, C)
